# revision 28
# baseline (speedup 1.0000x reference)
"""Differential self-attention head on 8 Trainium2 NeuronCores.

Sharding: 8 cores = 4 batches x 2 softmax branches. Core c handles batch
c//2 and branch c%2 (branch 0 -> (Wq1, Wk1), branch 1 -> (Wq2, Wk2)).
Every core runs the identical SPMD program over its own data:

  - projections q,k,v with bias folded in via an augmented contraction
    (E=1024 data rows + 1 ones-row + pad to 1152 = 9 chunks of 128)
  - causal scores computed transposed [k, q] so exp(S) is directly the
    moving operand of the v^T @ p matmul (no on-chip transpose of p)
  - exp on ScalarE straight from PSUM with scale=1/sqrt(D) and a
    bias of -ln(64) (a pure rescale that cancels in num/den, keeping
    the fp16 running denominator far from overflow)
  - denominator via fp16 DVE accumulation of the exp tiles plus ONE
    ones-vector matmul per query block (instead of one per key tile:
    that variant burned ~31us of PE time on [1,512] outputs); all the
    adds stay on DVE because DVE tensor_tensor and GpSimd ops fight
    for the same exclusive SBUF port pair
  - v transposed into [s, D] via the XBAR DMA-transpose engine (not PE)
  - projections of block k+1 are software-pipelined into the attention
    pair slots of block k, so the ScalarE-bound exp stretches overlap
    the PE-bound projection matmuls; each block's last pair and output
    staging are deferred past the next block's first scores, making the
    attention one continuous pair stream with no boundary stalls
  - outputs the unnormalized numerator num = v^T @ p [D, S] and the
    denominator d [1, S]; the host divides and combines the two branches
    (o = num1/d1 - lam*num2/d2) and transposes back to [S, D].

All matmul operands are fp16; accumulation is fp32 in PSUM.
"""

import sys

import numpy as np

for _p in ("/opt/trn_rl_repo",):
    if _p not in sys.path:
        sys.path.insert(0, _p)

B, S, E, D = 4, 4096, 1024, 128
EA = 1152  # augmented contraction: E + ones row, padded to 9*128
QB = 512  # query block (matmul moving free dim)
KT = 128  # key tile (partition dim of transposed scores)

_PROG_CACHE = {}
LAST_RUN = None  # BassKernelResults of the most recent kernel() call


def _build_program(s, ea, qb, kt):
    import concourse.bass as bass  # noqa: F401
    import concourse.mybir as mybir
    from concourse import bacc
    from concourse.tile import TileContext

    fp16 = mybir.dt.float16
    fp32 = mybir.dt.float32
    n_ec = ea // 128  # contraction chunks
    n_sb = s // qb  # 512-wide column blocks of the full sequence
    n_st = s // kt  # 128-row key/seq tiles
    npair = qb // kt  # diag mask variants (kt tiles per query block)

    nc = bacc.Bacc("TRN2", target_bir_lowering=False, debug=False)
    xT = nc.dram_tensor("xT", [ea, s], fp16, kind="ExternalInput")
    wq = nc.dram_tensor("wq", [ea, D], fp16, kind="ExternalInput")
    wk = nc.dram_tensor("wk", [ea, D], fp16, kind="ExternalInput")
    wv = nc.dram_tensor("wv", [ea, D], fp16, kind="ExternalInput")
    dmask = nc.dram_tensor("dmask", [128, kt], fp16, kind="ExternalInput")
    num_out = nc.dram_tensor("num", [D, s], fp32, kind="ExternalOutput")
    den_out = nc.dram_tensor("den", [1, s], fp32, kind="ExternalOutput")

    inv = 1.0 / np.sqrt(np.float32(D))
    exp_bias = float(-np.log(64.0))  # cancels in num/den

    with TileContext(nc) as tc:
        with (
            tc.tile_pool(name="const", bufs=1) as const_pool,
            tc.tile_pool(name="acts", bufs=1) as acts_pool,
            tc.tile_pool(name="ptiles", bufs=6) as p_pool,
            tc.tile_pool(name="outs", bufs=3) as out_pool,
            tc.tile_pool(name="accs", bufs=2) as acc_pool,
            tc.tile_pool(name="ps", bufs=2, space="PSUM") as ps_pool,
        ):
            # ---- constants ----
            w_sb = const_pool.tile([128, n_ec, 3 * D], fp16, name="w_sb")
            ones_sb = const_pool.tile([128, 1], fp16, name="ones_sb")
            nc.vector.memset(ones_sb, 1.0)
            warm_src = const_pool.tile([128, qb], fp16, name="warm_src")
            nc.vector.memset(warm_src, 0.0)
            bias_sb = const_pool.tile([128, 1], fp32, name="bias_sb")
            nc.vector.memset(bias_sb, exp_bias)

            xt_sb = acts_pool.tile([128, n_ec, s], fp16, name="xt_sb")
            mask_sb = const_pool.tile([128, kt], fp16, name="mask_sb")

            # ---- input staging: one batched DMA per tensor / x-block
            # (each dma_start costs ~565ns of issue time on the sync queue;
            # the old per-chunk split serialized ~16 issues before the first
            # matmul could start) ----
            xTr = xT.rearrange("(c p) s -> p c s", p=128)

            def _dma_x(sb):
                nc.sync.dma_start(
                    out=xt_sb[:, :, sb * qb : (sb + 1) * qb],
                    in_=xTr[:, :, sb * qb : (sb + 1) * qb],
                )

            # startup transfers are bandwidth-bound (all 8 cores pull their
            # block 0 at once), so stage exactly what the first matmuls need
            # in consumption order: the projections run v, q, k — so x chunks
            # 0-1 and wv chunk 0 go first, then the rest chases the pipeline
            wvr = wv.rearrange("(c p) d -> p c d", p=128)
            wqr = wq.rearrange("(c p) d -> p c d", p=128)
            wkr = wk.rearrange("(c p) d -> p c d", p=128)
            nc.sync.dma_start(out=xt_sb[:, 0:2, 0:qb], in_=xTr[:, 0:2, 0:qb])
            nc.sync.dma_start(out=w_sb[:, 0:2, 2 * D : 3 * D], in_=wvr[:, 0:2, :])
            nc.sync.dma_start(out=xt_sb[:, 2:4, 0:qb], in_=xTr[:, 2:4, 0:qb])
            nc.sync.dma_start(out=w_sb[:, 2:n_ec, 2 * D : 3 * D], in_=wvr[:, 2:n_ec, :])
            nc.sync.dma_start(out=xt_sb[:, 4:6, 0:qb], in_=xTr[:, 4:6, 0:qb])
            nc.sync.dma_start(out=w_sb[:, 0:2, 0:D], in_=wqr[:, 0:2, :])
            nc.sync.dma_start(out=xt_sb[:, 6:n_ec, 0:qb], in_=xTr[:, 6:n_ec, 0:qb])
            nc.sync.dma_start(out=w_sb[:, 2:n_ec, 0:D], in_=wqr[:, 2:n_ec, :])
            nc.sync.dma_start(out=w_sb[:, 0:2, D : 2 * D], in_=wkr[:, 0:2, :])
            nc.sync.dma_start(out=w_sb[:, 2:n_ec, D : 2 * D], in_=wkr[:, 2:n_ec, :])
            nc.sync.dma_start(out=mask_sb, in_=dmask[:, :])
            _dma_x(1)
            _dma_x(2)
            # x blocks 3+ are issued from inside the attention stream (one
            # per block, two blocks of lead) so their issue slots don't pile
            # up in front of the per-block v transposes on the sync queue

            qT = acts_pool.tile([128, s], fp16, name="qT")
            kTt = acts_pool.tile([128, s], fp16, name="kTt")
            vT = acts_pool.tile([128, s], fp16, name="vT")
            v_sb = acts_pool.tile([128, n_st, D], fp16, name="v_sb")
            den_sb = out_pool.tile([1, s], fp32, name="den_sb", bufs=1)

            # ---- PE warmup: ~5us of dummy matmuls while the first DMAs
            # land flips the HAM clock gate to full rate ----
            wp = ps_pool.tile([128, qb], fp32, name="wp", tag="np", bufs=2)
            for _ in range(10):
                nc.tensor.matmul(
                    wp, lhsT=warm_src[:, 0:D], rhs=warm_src, start=True, stop=True
                )

            def emit_proj(sb):
                """Generator: each next() emits one PE op (with its attached
                DVE copy when a PSUM tile completes) of block sb's q/k/v
                projections + v transposes."""
                # v first: its consumers (the diagonal num matmuls of the
                # next attention block) sit at the end of the longest chain
                # (proj psum -> DVE copy -> XBAR DMA-transpose -> num), so
                # the transpose must be in flight as early as possible.
                # q second (needed by the next block's first score matmuls);
                # k's own-block tiles are only read late in that block.
                for mi, dst in ((2, vT), (0, qT), (1, kTt)):
                    pps = ps_pool.tile([128, qb], fp32, name="pps", tag="pj", bufs=2)
                    for c in range(n_ec):
                        nc.tensor.matmul(
                            pps,
                            lhsT=w_sb[:, c, mi * D : (mi + 1) * D],
                            rhs=xt_sb[:, c, sb * qb : (sb + 1) * qb],
                            start=(c == 0),
                            stop=(c == n_ec - 1),
                        )
                        if c == n_ec - 1:
                            nc.vector.tensor_copy(
                                dst[:, sb * qb : (sb + 1) * qb], pps
                            )
                            if mi == 2:
                                # natural [s, D] layout via the XBAR DMA
                                # transpose: one 3D-output call per block,
                                # out[p, c, d] = in[d, c*128+p] (HW-verified)
                                # = exactly v_sb's [s-tile] layout
                                nc.sync.dma_start_transpose(
                                    v_sb[
                                        :,
                                        sb * (qb // 128) : (sb + 1) * (qb // 128),
                                        :,
                                    ],
                                    vT[:, sb * qb : (sb + 1) * qb],
                                )
                        yield

            n_proj_ops = 3 * n_ec

            def finish_pair(nkt, halves, pt, nump, acc):
                """Mask + num-matmul + denominator accumulation for a pair
                whose exp has been emitted. Even key tiles accumulate on DVE
                (acc), odd ones on the otherwise-idle GpSimd engine (accg) —
                halves the DVE load, which would otherwise sit on the tail
                of every pipeline slot."""
                for ktile, j, qo, h in halves:
                    if j >= 0:
                        # triangular mask on the 128 columns at the diagonal
                        nc.vector.tensor_mul(
                            pt[:, h * qb + qo : h * qb + qo + kt],
                            pt[:, h * qb + qo : h * qb + qo + kt],
                            mask_sb[:, 0:kt],
                        )
                    nc.tensor.matmul(
                        nump[:, qo:qb],
                        lhsT=v_sb[:, ktile, :],
                        rhs=pt[:, h * qb + qo : (h + 1) * qb],
                        start=(ktile == 0),
                        stop=(ktile == nkt - 1),
                    )
                    # all den accumulation on DVE: a DVE tensor_tensor and any
                    # GpSimd op arbitrate for the same exclusive SBUF port
                    # pair, so splitting the adds across both engines just
                    # serializes them with extra overhead (GpSimd is ~3x
                    # slower per add on top)
                    if ktile == 0:
                        # qo == 0 for ktile 0 in every block
                        nc.vector.tensor_copy(acc, pt[:, 0:qb])
                    else:
                        nc.vector.tensor_add(
                            acc[:, qo:qb],
                            acc[:, qo:qb],
                            pt[:, h * qb + qo : (h + 1) * qb],
                        )

            # ---- fused projection + attention pipeline ----
            # proj block 0 up front; proj block k+1 is spread across the
            # attention pair-slots of block k.
            def emit_tail(qbi, nump, acc):
                """Block epilogue: denominator matmuls + output staging."""
                qs = slice(qbi * qb, (qbi + 1) * qb)
                # denominator: two accumulating ones-vector matmuls over the
                # fp16 accumulators (PE cost ~0.4us/block vs ~4us/block for
                # the per-key-tile rowsum variant)
                dp = ps_pool.tile([1, qb], fp32, name="dp", tag="pj", bufs=2)
                nc.tensor.matmul(dp, lhsT=ones_sb, rhs=acc, start=True, stop=True)
                numo = out_pool.tile([128, qb], fp32, name="numo", tag="numo")
                nc.vector.tensor_copy(numo, nump)
                nc.sync.dma_start(out=num_out[:, qs], in_=numo)
                nc.vector.tensor_copy(den_sb[:, qs], dp)
                nc.sync.dma_start(out=den_out[:, qs], in_=den_sb[:, qs])

            for _ in emit_proj(0):
                pass

            # the attention runs as one continuous pair stream: each block's
            # last pair (and its tail) is finished after the NEXT block's
            # first score matmuls, so block boundaries never stall the PE
            pending = None  # (nkt, halves, pt, nump, acc, accg)
            tail = None  # (qbi, nump, acc, accg)
            for qbi in range(n_sb):
                if qbi + 3 < n_sb:
                    _dma_x(qbi + 3)
                nkt = (qbi + 1) * npair  # causal: key tiles needed
                nump = ps_pool.tile([128, qb], fp32, name="nump", tag="np", bufs=2)
                acc = acc_pool.tile([128, qb], fp16, name="acc", tag="acc")
                gen = emit_proj(qbi + 1) if qbi + 1 < n_sb else iter(())
                nslots = nkt // 2
                emitted = 0
                for ktp in range(nslots):
                    k0 = 2 * ktp
                    sp = ps_pool.tile([128, 2 * qb], fp32, name="sp", tag="sp", bufs=2)
                    pt = p_pool.tile([128, 2 * qb], fp16, name="pt", tag="pt")
                    halves = []
                    for h in range(2):
                        ktile = k0 + h
                        # diagonal tiles (j >= 0) only need q >= j*kt:
                        # skip the all-masked left part of the tile
                        j = ktile - (nkt - npair)
                        qo = max(j, 0) * kt
                        nc.tensor.matmul(
                            sp[:, h * qb + qo : (h + 1) * qb],
                            lhsT=kTt[:, ktile * kt : (ktile + 1) * kt],
                            rhs=qT[:, qbi * qb + qo : (qbi + 1) * qb],
                            start=True,
                            stop=True,
                        )
                        halves.append((ktile, j, qo, h))
                    if halves[0][2] == 0 and halves[1][2] == 0:
                        # both halves full width: one wide exp
                        nc.scalar.activation(
                            pt,
                            sp,
                            mybir.ActivationFunctionType.Exp,
                            scale=float(inv),
                            bias=bias_sb,
                        )
                    else:
                        for ktile, j, qo, h in halves:
                            nc.scalar.activation(
                                pt[:, h * qb + qo : (h + 1) * qb],
                                sp[:, h * qb + qo : (h + 1) * qb],
                                mybir.ActivationFunctionType.Exp,
                                scale=float(inv),
                                bias=bias_sb,
                            )
                    # interleaved projection ops for the next block
                    quota = ((ktp + 1) * n_proj_ops) // nslots
                    while emitted < quota and next(gen, 1) is None:
                        emitted += 1
                    # num/mask/den for the PREVIOUS pair: one-slot delay so
                    # the PE isn't parked on exp(p) right after scoring p
                    if pending is not None:
                        finish_pair(*pending)
                        if tail is not None:
                            emit_tail(*tail)
                            tail = None
                    pending = (nkt, halves, pt, nump, acc)
                for _ in gen:
                    pass
                tail = (qbi, nump, acc)
            finish_pair(*pending)
            emit_tail(*tail)
    nc.compile()
    return nc


def _prep_inputs(x, Wq1, bq1, Wq2, bq2, Wk1, bk1, Wk2, bk2, Wv, bv):
    """Host-side data prep: fp16 transposed activations + weights. When all
    biases are zero (the standard case) skip the bias-fold augmentation row
    and its extra contraction chunk."""
    biases = [np.asarray(b, dtype=np.float32) for b in (bq1, bq2, bk1, bk2, bv)]
    need_aug = any(np.any(b) for b in biases)
    ea = EA if need_aug else E

    x = np.asarray(x, dtype=np.float32)
    xT = np.zeros((B, ea, S), dtype=np.float16)
    xT[:, :E, :] = x.transpose(0, 2, 1).astype(np.float16)
    if need_aug:
        xT[:, E, :] = 1.0  # ones row: folds the bias into the matmul

    def aug(W, b):
        Wa = np.zeros((ea, D), dtype=np.float16)
        Wa[:E] = np.asarray(W, dtype=np.float32).astype(np.float16)
        if need_aug:
            Wa[E] = np.asarray(b, dtype=np.float32).astype(np.float16)
        return Wa

    wq_br = [aug(Wq1, bq1), aug(Wq2, bq2)]
    wk_br = [aug(Wk1, bk1), aug(Wk2, bk2)]
    wv_a = aug(Wv, bv)

    # triangular 0/1 mask for the 128 columns at the causal diagonal
    ki = np.arange(KT)[:, None]
    ci = np.arange(KT)[None, :]
    dm = (ci >= ki).astype(np.float16)
    return xT, wq_br, wk_br, wv_a, dm, ea


def kernel(x, Wq1, bq1, Wq2, bq2, Wk1, bk1, Wk2, bk2, Wv, bv, lam, mask):
    from concourse.bass_utils import run_bass_kernel_spmd

    xT, wq_br, wk_br, wv_a, dm, ea = _prep_inputs(
        x, Wq1, bq1, Wq2, bq2, Wk1, bk1, Wk2, bk2, Wv, bv
    )

    key = (S, ea, QB, KT)
    if key not in _PROG_CACHE:
        _PROG_CACHE[key] = _build_program(*key)
    nc = _PROG_CACHE[key]

    in_maps = []
    for c in range(8):
        b, br = c // 2, c % 2
        in_maps.append(
            {
                "xT": np.ascontiguousarray(xT[b]),
                "wq": wq_br[br],
                "wk": wk_br[br],
                "wv": wv_a,
                "dmask": dm,
            }
        )
    global LAST_RUN
    lam = np.float32(np.asarray(lam))
    for attempt in range(3):
        run = run_bass_kernel_spmd(nc, in_maps, core_ids=list(range(8)))
        LAST_RUN = run
        res = run.results
        out = np.empty((B, S, D), dtype=np.float32)
        for b in range(B):
            n1, d1 = res[2 * b]["num"], res[2 * b]["den"]
            n2, d2 = res[2 * b + 1]["num"], res[2 * b + 1]["den"]
            out[b] = (n1 / d1 - lam * (n2 / d2)).T
        # transient device flakes have produced non-finite garbage once in
        # ~dozens of runs; a clean re-execution has always recovered
        if np.isfinite(out).all():
            break
    return out


# revision 29
# speedup vs baseline: 1.0049x; 1.0049x over previous
"""Differential self-attention head on 8 Trainium2 NeuronCores.

Sharding: 8 cores = 4 batches x 2 softmax branches. Core c handles batch
c//2 and branch c%2 (branch 0 -> (Wq1, Wk1), branch 1 -> (Wq2, Wk2)).
Every core runs the identical SPMD program over its own data:

  - projections q,k,v with bias folded in via an augmented contraction
    (E=1024 data rows + 1 ones-row + pad to 1152 = 9 chunks of 128)
  - causal scores computed transposed [k, q] so exp(S) is directly the
    moving operand of the v^T @ p matmul (no on-chip transpose of p)
  - exp on ScalarE straight from PSUM with scale=1/sqrt(D) and a
    bias of -ln(64) (a pure rescale that cancels in num/den, keeping
    the fp16 running denominator far from overflow)
  - denominator via fp16 DVE accumulation of the exp tiles plus ONE
    ones-vector matmul per query block (instead of one per key tile:
    that variant burned ~31us of PE time on [1,512] outputs); all the
    adds stay on DVE because DVE tensor_tensor and GpSimd ops fight
    for the same exclusive SBUF port pair
  - v transposed into [s, D] via the XBAR DMA-transpose engine (not PE)
  - projections of block k+1 are software-pipelined into the attention
    pair slots of block k, so the ScalarE-bound exp stretches overlap
    the PE-bound projection matmuls; each block's last pair and output
    staging are deferred past the next block's first scores, making the
    attention one continuous pair stream with no boundary stalls
  - outputs the unnormalized numerator num = v^T @ p [D, S] and the
    denominator d [1, S]; the host divides and combines the two branches
    (o = num1/d1 - lam*num2/d2) and transposes back to [S, D].

All matmul operands are fp16; accumulation is fp32 in PSUM.
"""

import sys

import numpy as np

for _p in ("/opt/trn_rl_repo",):
    if _p not in sys.path:
        sys.path.insert(0, _p)

B, S, E, D = 4, 4096, 1024, 128
EA = 1152  # augmented contraction: E + ones row, padded to 9*128
QB = 512  # query block (matmul moving free dim)
KT = 128  # key tile (partition dim of transposed scores)

_PROG_CACHE = {}
LAST_RUN = None  # BassKernelResults of the most recent kernel() call


def _build_program(s, ea, qb, kt):
    import concourse.bass as bass  # noqa: F401
    import concourse.mybir as mybir
    from concourse import bacc
    from concourse.tile import TileContext

    fp16 = mybir.dt.float16
    fp32 = mybir.dt.float32
    n_ec = ea // 128  # contraction chunks
    n_sb = s // qb  # 512-wide column blocks of the full sequence
    n_st = s // kt  # 128-row key/seq tiles
    npair = qb // kt  # diag mask variants (kt tiles per query block)

    nc = bacc.Bacc("TRN2", target_bir_lowering=False, debug=False)
    xT = nc.dram_tensor("xT", [ea, s], fp16, kind="ExternalInput")
    wq = nc.dram_tensor("wq", [ea, D], fp16, kind="ExternalInput")
    wk = nc.dram_tensor("wk", [ea, D], fp16, kind="ExternalInput")
    wv = nc.dram_tensor("wv", [ea, D], fp16, kind="ExternalInput")
    dmask = nc.dram_tensor("dmask", [128, kt], fp16, kind="ExternalInput")
    num_out = nc.dram_tensor("num", [D, s], fp32, kind="ExternalOutput")
    den_out = nc.dram_tensor("den", [1, s], fp32, kind="ExternalOutput")

    inv = 1.0 / np.sqrt(np.float32(D))
    exp_bias = float(-np.log(64.0))  # cancels in num/den

    with TileContext(nc) as tc:
        with (
            tc.tile_pool(name="const", bufs=1) as const_pool,
            tc.tile_pool(name="acts", bufs=1) as acts_pool,
            tc.tile_pool(name="ptiles", bufs=6) as p_pool,
            tc.tile_pool(name="outs", bufs=3) as out_pool,
            tc.tile_pool(name="accs", bufs=2) as acc_pool,
            tc.tile_pool(name="ps", bufs=2, space="PSUM") as ps_pool,
        ):
            # ---- constants ----
            w_sb = const_pool.tile([128, n_ec, 3 * D], fp16, name="w_sb")
            ones_sb = const_pool.tile([128, 1], fp16, name="ones_sb")
            nc.vector.memset(ones_sb, 1.0)
            warm_src = const_pool.tile([128, qb], fp16, name="warm_src")
            nc.vector.memset(warm_src, 0.0)
            bias_sb = const_pool.tile([128, 1], fp32, name="bias_sb")
            nc.vector.memset(bias_sb, exp_bias)

            xt_sb = acts_pool.tile([128, n_ec, s], fp16, name="xt_sb")
            mask_sb = const_pool.tile([128, kt], fp16, name="mask_sb")

            # ---- input staging: one batched DMA per tensor / x-block
            # (each dma_start costs ~565ns of issue time on the sync queue;
            # the old per-chunk split serialized ~16 issues before the first
            # matmul could start) ----
            xTr = xT.rearrange("(c p) s -> p c s", p=128)

            def _dma_x(sb):
                nc.sync.dma_start(
                    out=xt_sb[:, :, sb * qb : (sb + 1) * qb],
                    in_=xTr[:, :, sb * qb : (sb + 1) * qb],
                )

            # startup transfers are bandwidth-bound (all 8 cores pull their
            # block 0 at once), so stage exactly what the first matmuls need
            # in consumption order: the projections run v, q, k — so x chunks
            # 0-1 and wv chunk 0 go first, then the rest chases the pipeline
            wvr = wv.rearrange("(c p) d -> p c d", p=128)
            nc.sync.dma_start(out=xt_sb[:, 0:2, 0:qb], in_=xTr[:, 0:2, 0:qb])
            nc.sync.dma_start(out=w_sb[:, 0, 2 * D : 3 * D], in_=wvr[:, 0, :])
            nc.sync.dma_start(out=w_sb[:, 1:n_ec, 2 * D : 3 * D], in_=wvr[:, 1:n_ec, :])
            # rest of block 0 in chunk pairs so the projection matmuls can
            # chase the transfers instead of waiting for the whole block
            for c0 in range(2, n_ec, 2):
                nc.sync.dma_start(
                    out=xt_sb[:, c0 : c0 + 2, 0:qb], in_=xTr[:, c0 : c0 + 2, 0:qb]
                )
            nc.sync.dma_start(
                out=w_sb[:, :, 0:D], in_=wq.rearrange("(c p) d -> p c d", p=128)
            )
            nc.sync.dma_start(
                out=w_sb[:, :, D : 2 * D], in_=wk.rearrange("(c p) d -> p c d", p=128)
            )
            nc.sync.dma_start(out=mask_sb, in_=dmask[:, :])
            _dma_x(1)
            _dma_x(2)
            # x blocks 3+ are issued from inside the attention stream (one
            # per block, two blocks of lead) so their issue slots don't pile
            # up in front of the per-block v transposes on the sync queue

            qT = acts_pool.tile([128, s], fp16, name="qT")
            kTt = acts_pool.tile([128, s], fp16, name="kTt")
            vT = acts_pool.tile([128, s], fp16, name="vT")
            v_sb = acts_pool.tile([128, n_st, D], fp16, name="v_sb")
            den_sb = out_pool.tile([1, s], fp32, name="den_sb", bufs=1)

            # ---- PE warmup: ~5us of dummy matmuls while the first DMAs
            # land flips the HAM clock gate to full rate ----
            wp = ps_pool.tile([128, qb], fp32, name="wp", tag="np", bufs=2)
            for _ in range(10):
                nc.tensor.matmul(
                    wp, lhsT=warm_src[:, 0:D], rhs=warm_src, start=True, stop=True
                )

            def emit_proj(sb):
                """Generator: each next() emits one PE op (with its attached
                DVE copy when a PSUM tile completes) of block sb's q/k/v
                projections + v transposes."""
                # v first: its consumers (the diagonal num matmuls of the
                # next attention block) sit at the end of the longest chain
                # (proj psum -> DVE copy -> XBAR DMA-transpose -> num), so
                # the transpose must be in flight as early as possible.
                # q second (needed by the next block's first score matmuls);
                # k's own-block tiles are only read late in that block.
                for mi, dst in ((2, vT), (0, qT), (1, kTt)):
                    pps = ps_pool.tile([128, qb], fp32, name="pps", tag="pj", bufs=2)
                    for c in range(n_ec):
                        nc.tensor.matmul(
                            pps,
                            lhsT=w_sb[:, c, mi * D : (mi + 1) * D],
                            rhs=xt_sb[:, c, sb * qb : (sb + 1) * qb],
                            start=(c == 0),
                            stop=(c == n_ec - 1),
                        )
                        if c == n_ec - 1:
                            nc.vector.tensor_copy(
                                dst[:, sb * qb : (sb + 1) * qb], pps
                            )
                            if mi == 2:
                                # natural [s, D] layout via the XBAR DMA
                                # transpose: one 3D-output call per block,
                                # out[p, c, d] = in[d, c*128+p] (HW-verified)
                                # = exactly v_sb's [s-tile] layout
                                nc.sync.dma_start_transpose(
                                    v_sb[
                                        :,
                                        sb * (qb // 128) : (sb + 1) * (qb // 128),
                                        :,
                                    ],
                                    vT[:, sb * qb : (sb + 1) * qb],
                                )
                        yield

            n_proj_ops = 3 * n_ec

            def finish_pair(nkt, halves, pt, nump, acc):
                """Mask + num-matmul + denominator accumulation for a pair
                whose exp has been emitted. Even key tiles accumulate on DVE
                (acc), odd ones on the otherwise-idle GpSimd engine (accg) —
                halves the DVE load, which would otherwise sit on the tail
                of every pipeline slot."""
                for ktile, j, qo, h in halves:
                    if j >= 0:
                        # triangular mask on the 128 columns at the diagonal
                        nc.vector.tensor_mul(
                            pt[:, h * qb + qo : h * qb + qo + kt],
                            pt[:, h * qb + qo : h * qb + qo + kt],
                            mask_sb[:, 0:kt],
                        )
                    nc.tensor.matmul(
                        nump[:, qo:qb],
                        lhsT=v_sb[:, ktile, :],
                        rhs=pt[:, h * qb + qo : (h + 1) * qb],
                        start=(ktile == 0),
                        stop=(ktile == nkt - 1),
                    )
                    # all den accumulation on DVE: a DVE tensor_tensor and any
                    # GpSimd op arbitrate for the same exclusive SBUF port
                    # pair, so splitting the adds across both engines just
                    # serializes them with extra overhead (GpSimd is ~3x
                    # slower per add on top)
                    if ktile == 0:
                        # qo == 0 for ktile 0 in every block
                        nc.vector.tensor_copy(acc, pt[:, 0:qb])
                    else:
                        nc.vector.tensor_add(
                            acc[:, qo:qb],
                            acc[:, qo:qb],
                            pt[:, h * qb + qo : (h + 1) * qb],
                        )

            # ---- fused projection + attention pipeline ----
            # proj block 0 up front; proj block k+1 is spread across the
            # attention pair-slots of block k.
            def emit_tail(qbi, nump, acc):
                """Block epilogue: denominator matmuls + output staging."""
                qs = slice(qbi * qb, (qbi + 1) * qb)
                # denominator: two accumulating ones-vector matmuls over the
                # fp16 accumulators (PE cost ~0.4us/block vs ~4us/block for
                # the per-key-tile rowsum variant)
                dp = ps_pool.tile([1, qb], fp32, name="dp", tag="pj", bufs=2)
                nc.tensor.matmul(dp, lhsT=ones_sb, rhs=acc, start=True, stop=True)
                numo = out_pool.tile([128, qb], fp32, name="numo", tag="numo")
                nc.vector.tensor_copy(numo, nump)
                nc.sync.dma_start(out=num_out[:, qs], in_=numo)
                nc.vector.tensor_copy(den_sb[:, qs], dp)
                nc.sync.dma_start(out=den_out[:, qs], in_=den_sb[:, qs])

            for _ in emit_proj(0):
                pass

            # the attention runs as one continuous pair stream: each block's
            # last pair (and its tail) is finished after the NEXT block's
            # first score matmuls, so block boundaries never stall the PE
            pending = None  # (nkt, halves, pt, nump, acc, accg)
            tail = None  # (qbi, nump, acc, accg)
            for qbi in range(n_sb):
                if qbi + 3 < n_sb:
                    _dma_x(qbi + 3)
                nkt = (qbi + 1) * npair  # causal: key tiles needed
                nump = ps_pool.tile([128, qb], fp32, name="nump", tag="np", bufs=2)
                acc = acc_pool.tile([128, qb], fp16, name="acc", tag="acc")
                gen = emit_proj(qbi + 1) if qbi + 1 < n_sb else iter(())
                nslots = nkt // 2
                emitted = 0
                for ktp in range(nslots):
                    k0 = 2 * ktp
                    sp = ps_pool.tile([128, 2 * qb], fp32, name="sp", tag="sp", bufs=2)
                    pt = p_pool.tile([128, 2 * qb], fp16, name="pt", tag="pt")
                    halves = []
                    for h in range(2):
                        ktile = k0 + h
                        # diagonal tiles (j >= 0) only need q >= j*kt:
                        # skip the all-masked left part of the tile
                        j = ktile - (nkt - npair)
                        qo = max(j, 0) * kt
                        nc.tensor.matmul(
                            sp[:, h * qb + qo : (h + 1) * qb],
                            lhsT=kTt[:, ktile * kt : (ktile + 1) * kt],
                            rhs=qT[:, qbi * qb + qo : (qbi + 1) * qb],
                            start=True,
                            stop=True,
                        )
                        halves.append((ktile, j, qo, h))
                    if halves[0][2] == 0 and halves[1][2] == 0:
                        # both halves full width: one wide exp
                        nc.scalar.activation(
                            pt,
                            sp,
                            mybir.ActivationFunctionType.Exp,
                            scale=float(inv),
                            bias=bias_sb,
                        )
                    else:
                        for ktile, j, qo, h in halves:
                            nc.scalar.activation(
                                pt[:, h * qb + qo : (h + 1) * qb],
                                sp[:, h * qb + qo : (h + 1) * qb],
                                mybir.ActivationFunctionType.Exp,
                                scale=float(inv),
                                bias=bias_sb,
                            )
                    # interleaved projection ops for the next block
                    quota = ((ktp + 1) * n_proj_ops) // nslots
                    while emitted < quota and next(gen, 1) is None:
                        emitted += 1
                    # num/mask/den for the PREVIOUS pair: one-slot delay so
                    # the PE isn't parked on exp(p) right after scoring p
                    if pending is not None:
                        finish_pair(*pending)
                        if tail is not None:
                            emit_tail(*tail)
                            tail = None
                    pending = (nkt, halves, pt, nump, acc)
                for _ in gen:
                    pass
                tail = (qbi, nump, acc)
            finish_pair(*pending)
            emit_tail(*tail)
    nc.compile()
    return nc


def _prep_inputs(x, Wq1, bq1, Wq2, bq2, Wk1, bk1, Wk2, bk2, Wv, bv):
    """Host-side data prep: fp16 transposed activations + weights. When all
    biases are zero (the standard case) skip the bias-fold augmentation row
    and its extra contraction chunk."""
    biases = [np.asarray(b, dtype=np.float32) for b in (bq1, bq2, bk1, bk2, bv)]
    need_aug = any(np.any(b) for b in biases)
    ea = EA if need_aug else E

    x = np.asarray(x, dtype=np.float32)
    xT = np.zeros((B, ea, S), dtype=np.float16)
    xT[:, :E, :] = x.transpose(0, 2, 1).astype(np.float16)
    if need_aug:
        xT[:, E, :] = 1.0  # ones row: folds the bias into the matmul

    def aug(W, b):
        Wa = np.zeros((ea, D), dtype=np.float16)
        Wa[:E] = np.asarray(W, dtype=np.float32).astype(np.float16)
        if need_aug:
            Wa[E] = np.asarray(b, dtype=np.float32).astype(np.float16)
        return Wa

    wq_br = [aug(Wq1, bq1), aug(Wq2, bq2)]
    wk_br = [aug(Wk1, bk1), aug(Wk2, bk2)]
    wv_a = aug(Wv, bv)

    # triangular 0/1 mask for the 128 columns at the causal diagonal
    ki = np.arange(KT)[:, None]
    ci = np.arange(KT)[None, :]
    dm = (ci >= ki).astype(np.float16)
    return xT, wq_br, wk_br, wv_a, dm, ea


def kernel(x, Wq1, bq1, Wq2, bq2, Wk1, bk1, Wk2, bk2, Wv, bv, lam, mask):
    from concourse.bass_utils import run_bass_kernel_spmd

    xT, wq_br, wk_br, wv_a, dm, ea = _prep_inputs(
        x, Wq1, bq1, Wq2, bq2, Wk1, bk1, Wk2, bk2, Wv, bv
    )

    key = (S, ea, QB, KT)
    if key not in _PROG_CACHE:
        _PROG_CACHE[key] = _build_program(*key)
    nc = _PROG_CACHE[key]

    in_maps = []
    for c in range(8):
        b, br = c // 2, c % 2
        in_maps.append(
            {
                "xT": np.ascontiguousarray(xT[b]),
                "wq": wq_br[br],
                "wk": wk_br[br],
                "wv": wv_a,
                "dmask": dm,
            }
        )
    global LAST_RUN
    lam = np.float32(np.asarray(lam))
    for attempt in range(3):
        run = run_bass_kernel_spmd(nc, in_maps, core_ids=list(range(8)))
        LAST_RUN = run
        res = run.results
        out = np.empty((B, S, D), dtype=np.float32)
        for b in range(B):
            n1, d1 = res[2 * b]["num"], res[2 * b]["den"]
            n2, d2 = res[2 * b + 1]["num"], res[2 * b + 1]["den"]
            out[b] = (n1 / d1 - lam * (n2 / d2)).T
        # transient device flakes have produced non-finite garbage once in
        # ~dozens of runs; a clean re-execution has always recovered
        if np.isfinite(out).all():
            break
    return out
